# revision 10
# baseline (speedup 1.0000x reference)
"""Multi-head causal attention block on 8 TRN2 NeuronCores.

Sharding: batch b = core//4 (2 groups of 4 cores), heads = 4*(core%4)..+3
within the group (tensor parallel over heads). Host pre-slices/permutes/
bf16-casts the weights and pre-transposes X.

Per core (4 heads, processed as 2 pairs):
  Q^T, K^T = wqk^T @ X^T   [head-pair packed: rows 0:64 head even, 64:128 odd]
  V_aug    = X @ [Wv|1|0]  (65th col per head is constant 1 -> rowsums)
  scores^T(kb) = K_h^T.T @ Q_h^T  ->  [k, q] layout, causal triangle only
  probs^T = exp(scores^T/8) via ACT straight into SBUF; diag block's upper
      triangle zeroed post-exp on GpSimd (logits are small, no overflow)
  attn^T accumulation ("flipped AV"): for each k-block j,
      psum_c[ch 0:65, q] += V_aug_j.T @ probsT_j  -- V is the stationary
      operand so there is one small LDWEIGHTS per k-block instead of one
      128x128 probs reload per output block, and the result lands directly
      in merged^T [ch, q] layout (no PE transposes). Row 64 accumulates the
      softmax denominator.
  normalize: recip(row 64) on DVE, partition-broadcast on GpSimd,
      multiply into mts^T on DVE.
  A2A within each 4-core group (replica_groups [[0..3],[4..7]]), one per
      head pair, overlapped with the other pair's attention / projection
      pass 1; then 2-pass row-parallel projection + bias, DMA out.
"""

import os
import sys

import numpy as np

if "/opt/trn_rl_repo" not in sys.path:
    sys.path.insert(0, "/opt/trn_rl_repo")

S = 2048
D = 1024
H = 16
HD = 64
NCORES = 8
SQ = S // 4   # rows of output per core
NKB = S // 128  # 16 k blocks per head
NPRE = 8      # scores steps interleaved into the V phase

_NC_CACHE = {}


def _build_nc(debug_taps=False):
    import concourse.bass as bass
    import concourse.mybir as mybir
    import concourse.tile as tile
    from concourse import bacc

    f32 = mybir.dt.float32
    bf16 = mybir.dt.bfloat16

    nc = bacc.Bacc("TRN2", target_bir_lowering=False, debug=False,
                   num_devices=NCORES)

    xt_p = nc.dram_tensor("xt", [D, S], bf16, kind="ExternalInput")
    wqk_p = nc.dram_tensor("wqk", [D, 512], bf16, kind="ExternalInput")
    wv_p = nc.dram_tensor("wv", [D, 264], bf16, kind="ExternalInput")
    wp_p = nc.dram_tensor("wp", [D, D], bf16, kind="ExternalInput")
    bqk_p = nc.dram_tensor("bqk", [128, 4], f32, kind="ExternalInput")
    bv_p = nc.dram_tensor("bv", [1, 264], f32, kind="ExternalInput")
    bp_p = nc.dram_tensor("bp", [1, D], f32, kind="ExternalInput")
    gm_p = nc.dram_tensor("gmask", [128, 8], f32, kind="ExternalInput")
    out_p = nc.dram_tensor("out", [SQ, D], f32, kind="ExternalOutput")
    dbg = {}
    if debug_taps:
        dbg["qk"] = nc.dram_tensor("dbg_qk", [4, 128, S], bf16,
                                   kind="ExternalOutput")
        dbg["v"] = nc.dram_tensor("dbg_v", [128, 16 * 264], bf16,
                                  kind="ExternalOutput")
        dbg["mt"] = nc.dram_tensor("dbg_mt", [2, 128, S], bf16,
                                   kind="ExternalOutput")
        dbg["pi"] = nc.dram_tensor("dbg_pi", [8, 128, 512], bf16,
                                   kind="ExternalOutput")

    EXP = mybir.ActivationFunctionType.Exp

    with tile.TileContext(nc, pool_alloc_mode="queue") as tc:
        with tc.tile_pool(name="pers", bufs=1) as pers, \
             tc.tile_pool(name="dram", bufs=1, space="DRAM") as dram:
            # ---- constants ----
            bqk_sb = pers.tile([128, 4], f32, tag="bqk", name="bqk")
            bv_row = pers.tile([1, 264], f32, tag="bvr", name="bvr")
            bp_row = pers.tile([1, D], f32, tag="bpr", name="bpr")
            bv_bc = pers.tile([128, 264], f32, tag="bvb", name="bvb")
            bp_bc = pers.tile([128, D], f32, tag="bpb", name="bpb")
            gm_sb = pers.tile([128, 8], f32, tag="gm", name="gm")

            # ---- persistent big tiles ----
            # Q head-pair packed: rows 0:64 head 2p, rows 64:128 head 2p+1
            qps = [pers.tile([128, S], bf16, tag=f"qp{p}", name=f"qp{p}")
                   for p in range(2)]
            # K per head, zero-padded to 128 rows (head ch in rows
            # (i%2)*64..+64, zeros elsewhere) so the scores matmul can
            # contract the full pair-packed Q partitions
            kpads = [pers.tile([128, S], bf16, tag=f"kp{i}", name=f"kp{i}")
                     for i in range(4)]
            # V_aug: 16 s-blocks x [128, 264]; head i at cols 66i..66i+63,
            # col 66i+64 is the constant-one column, 66i+65 zero pad
            vt = pers.tile([128, 16 * 264], bf16, tag="vt", name="vt")
            # merged^T per pair [128 ch, S]
            mts = [pers.tile([128, S], bf16, tag=f"mt{p}", name=f"mt{p}")
                   for p in range(2)]
            pis = [pers.tile([128, 512], bf16, tag=f"pi{i}", name=f"pi{i}")
                   for i in range(8)]

            for i in range(4):
                z = slice(64, 128) if i % 2 == 0 else slice(0, 64)
                nc.gpsimd.memset(kpads[i][z, :], 0.0)

            a2a_in = [dram.tile([8, 128, 512], bf16, tag=f"a2ai{p}",
                                name=f"a2ai{p}") for p in range(2)]
            a2a_out = [dram.tile([8, 128, 512], bf16, tag=f"a2ao{p}",
                                 name=f"a2ao{p}") for p in range(2)]

            with tc.tile_pool(name="pssc", bufs=2, space="PSUM") as pssc, \
                 tc.tile_pool(name="probs", bufs=NPRE + 2) as probs_pool, \
                 tc.tile_pool(name="small", bufs=4) as small, \
                 tc.tile_pool(name="bcp", bufs=4) as bcp, \
                 tc.tile_pool(name="stage", bufs=4) as stage_pool, \
                 tc.tile_pool(name="pj", bufs=4) as pj_pool:

                # steps: head-major; head i = 2*pr + hl
                steps = [(pr, hl, kb) for pr in range(2) for hl in range(2)
                         for kb in range(NKB)]
                NS = len(steps)
                ptiles = {}

                def emit_scores(pr, hl, kb):
                    Q = qps[pr]
                    K = kpads[2 * pr + hl]
                    qw = S - 128 * kb
                    q0 = 128 * kb
                    pT = probs_pool.tile([128, S], bf16, tag="pT",
                                         name=f"pT{pr}{hl}{kb}")
                    off = 0
                    while off < qw:
                        w = min(1024, qw - off)
                        ps = pssc.tile([128, 1024], f32, tag="sc",
                                       name="sc")
                        for c0 in range(0, w, 512):
                            cw = min(512, w - c0)
                            o = off + c0
                            nc.tensor.matmul(
                                ps[:, c0:c0 + cw],
                                K[:, q0:q0 + 128],
                                Q[:, q0 + o:q0 + o + cw],
                                start=True, stop=True)
                        nc.scalar.activation(
                            pT[:, off:off + w], ps[:, 0:w], EXP,
                            scale=0.125)
                        if off == 0:
                            # causal mask: zero the diag block's upper
                            # triangle post-exp on the idle Pool engine
                            nc.gpsimd.affine_select(
                                out=pT[:, 0:128], in_=pT[:, 0:128],
                                compare_op=mybir.AluOpType.is_ge,
                                fill=0.0, base=0,
                                pattern=[[1, 128]], channel_multiplier=-1)
                        off += w
                    ptiles[(pr, hl, kb)] = pT

                # ================= phase 1: QKV projections ==========
                with tc.tile_pool(name="ph1", bufs=1) as ph1:
                    xts = [ph1.tile([128, S], bf16, tag=f"xt{i}",
                                    name=f"xt{i}") for i in range(8)]
                    wqks = [ph1.tile([128, 512], bf16, tag=f"wqk{i}",
                                     name=f"wqk{i}") for i in range(8)]
                    wvs = [ph1.tile([128, 264], bf16, tag=f"wv{i}",
                                    name=f"wv{i}") for i in range(8)]

                    def in_eng(kb):
                        return nc.sync if kb % 2 == 0 else nc.scalar
                    for kb in range(8):
                        in_eng(kb).dma_start(
                            out=wqks[kb][:],
                            in_=wqk_p[kb * 128:(kb + 1) * 128, :])
                        in_eng(kb + 1).dma_start(
                            out=xts[kb][:, 0:512],
                            in_=xt_p[kb * 128:(kb + 1) * 128, 0:512])
                    for n2 in range(1, 4):
                        for kb in range(8):
                            in_eng(kb).dma_start(
                                out=xts[kb][:, n2 * 512:(n2 + 1) * 512],
                                in_=xt_p[kb * 128:(kb + 1) * 128,
                                         n2 * 512:(n2 + 1) * 512])
                    for kb in range(8):
                        nc.gpsimd.dma_start(
                            out=wvs[kb][:],
                            in_=wv_p[kb * 128:(kb + 1) * 128, :])
                    nc.scalar.dma_start(out=bqk_sb[:], in_=bqk_p[:])
                    nc.scalar.dma_start(out=bv_row[:], in_=bv_p[:])
                    nc.scalar.dma_start(out=bp_row[:], in_=bp_p[:])
                    nc.scalar.dma_start(out=gm_sb[:], in_=gm_p[:])
                    nc.gpsimd.partition_broadcast(bv_bc[:], bv_row[:])
                    nc.gpsimd.partition_broadcast(bp_bc[:], bp_row[:])

                    # ---- QK^T: kb-outer chases the DMA arrival; the 4
                    # m-blocks accumulate in 4 PSUM banks in parallel ----
                    with tc.tile_pool(name="psq", bufs=1,
                                      space="PSUM") as psq:
                        for n2 in range(4):
                            w0 = slice(n2 * 512, (n2 + 1) * 512)
                            pss = [psq.tile([128, 512], f32, tag=f"q{m}",
                                            name=f"q{m}") for m in range(4)]
                            for kb in range(8):
                                for m in range(4):
                                    nc.tensor.matmul(
                                        pss[m][:],
                                        wqks[kb][:, m * 128:(m + 1) * 128],
                                        xts[kb][:, w0],
                                        start=(kb == 0), stop=(kb == 7))
                            # evacuate on DVE (adds the per-row bias)
                            for m in range(2):
                                nc.vector.tensor_scalar_add(
                                    qps[m][:, w0], pss[m][:],
                                    bqk_sb[:, m:m + 1])
                            for m in range(2, 4):
                                ka = kpads[2 * (m - 2)]
                                kb2 = kpads[2 * (m - 2) + 1]
                                nc.vector.tensor_scalar_add(
                                    ka[0:64, w0], pss[m][0:64, :],
                                    bqk_sb[0:64, m:m + 1])
                                nc.vector.tensor_scalar_add(
                                    kb2[64:128, w0], pss[m][64:128, :],
                                    bqk_sb[64:128, m:m + 1])

                    # ---- V (+ ones column); prescores interleave ----
                    pre_iter = iter(range(NPRE))

                    def emit_pre():
                        s = next(pre_iter, None)
                        if s is not None:
                            emit_scores(*steps[s])

                    with tc.tile_pool(name="psv", bufs=2,
                                      space="PSUM") as psv:
                        for sb in range(16):
                            psvt = psv.tile([128, 264], f32, tag="v",
                                            name="v")
                            for kb in range(8):
                                nc.tensor.matmul(
                                    psvt[:],
                                    xts[kb][:, sb * 128:(sb + 1) * 128],
                                    wvs[kb][:],
                                    start=(kb == 0), stop=(kb == 7))
                            nc.vector.tensor_add(
                                vt[:, sb * 264:(sb + 1) * 264], psvt[:],
                                bv_bc[:])
                            if sb % 2 == 1:
                                emit_pre()
                        for _ in range(NPRE):
                            emit_pre()

                if debug_taps:
                    for p in range(2):
                        nc.sync.dma_start(out=dbg["qk"][p], in_=qps[p][:])
                        nc.sync.dma_start(out=dbg["qk"][2 + p][0:64],
                                          in_=kpads[2 * p][0:64, :])
                        nc.sync.dma_start(out=dbg["qk"][2 + p][64:128],
                                          in_=kpads[2 * p + 1][64:128, :])
                    nc.sync.dma_start(out=dbg["v"][:], in_=vt[:])

                # ============ attention steady state + a2a + proj =======
                with tc.tile_pool(name="wpp", bufs=1) as wpp, \
                     tc.tile_pool(name="psav", bufs=1,
                                  space="PSUM") as psav:
                    wps = [wpp.tile([128, D], bf16, tag=f"wp{i}",
                                    name=f"wp{i}") for i in range(8)]
                    avts = {}

                    def emit_av(pr, hl, j):
                        """attn^T accumulation for k-block j of head
                        i=2pr+hl: psum_c[0:65, :] += V_aug_j^T @ probsT_j.
                        Row 64 accumulates the softmax denominator."""
                        i = 2 * pr + hl
                        pT = ptiles.pop((pr, hl, j))
                        if j == 0:
                            for c in range(4):
                                avts[c] = psav.tile([128, 512], f32,
                                                    tag=f"av{c}",
                                                    name=f"av{c}")
                        vsl = vt[:, j * 264 + i * 66:j * 264 + i * 66 + 65]
                        for c in range(j // 4, 4):
                            qlo = max(512 * c, 128 * j)
                            qhi = 512 * (c + 1)
                            nc.tensor.matmul(
                                avts[c][0:65, qlo - 512 * c:512],
                                vsl,
                                pT[:, qlo - 128 * j:qhi - 128 * j],
                                start=(j == 0), stop=(j == 4 * c + 3))

                    def emit_norm(pr, hl, c):
                        """normalize chunk c of head (pr,hl) into mts."""
                        pa = avts[c]
                        rec = small.tile([1, 512], f32, tag="rec",
                                         name="rec")
                        nc.vector.reciprocal(rec[:], pa[64:65, :])
                        bc = bcp.tile([64, 512], f32, tag="bc", name="bc")
                        nc.gpsimd.partition_broadcast(bc[:], rec[:])
                        rows = slice(hl * 64, hl * 64 + 64)
                        nc.vector.tensor_mul(
                            mts[pr][rows, c * 512:(c + 1) * 512],
                            pa[0:64, :], bc[:])

                    def emit_stage(pr, gq):
                        # gmask zeroes the cross-group copy so the
                        # receiver's group-half add picks the in-group
                        # block (the slot index depends on the core's
                        # group, which only the per-core gmask data knows)
                        for d in (gq, gq + 4):
                            st = stage_pool.tile([128, 512], bf16,
                                                 tag="st", name="st")
                            nc.vector.tensor_scalar_mul(
                                st[:],
                                mts[pr][:, gq * 512:(gq + 1) * 512],
                                gm_sb[:, d:d + 1])
                            nc.sync.dma_start(out=a2a_in[pr][d], in_=st[:])

                    def emit_a2a(pr):
                        nc.gpsimd.collective_compute(
                            "AllToAll",
                            mybir.AluOpType.bypass,
                            replica_groups=[list(range(NCORES))],
                            ins=[a2a_in[pr][:].opt()],
                            outs=[a2a_out[pr][:].opt()])

                    def load_wp():
                        for kb in range(8):
                            eng = nc.sync if kb % 2 == 0 else nc.scalar
                            eng.dma_start(
                                out=wps[kb][:],
                                in_=wp_p[kb * 128:(kb + 1) * 128, :])

                    def recv(pr, jj, eng):
                        # receiver: sum the two group halves (one is zeros)
                        ta = stage_pool.tile([128, 512], bf16,
                                             tag="st", name="ca")
                        tb = stage_pool.tile([128, 512], bf16,
                                             tag="st", name="cb")
                        eng.dma_start(out=ta[:], in_=a2a_out[pr][jj])
                        eng.dma_start(out=tb[:], in_=a2a_out[pr][4 + jj])
                        nc.vector.tensor_add(pis[pr * 4 + jj][:],
                                             ta[:], tb[:])

                    # steady state: scores lead AV by NPRE steps (the
                    # prescores emitted during the V phase keep the lead)
                    for t in range(NPRE, NS + NPRE):
                        if t < NS:
                            emit_scores(*steps[t])
                        pr, hl, j = steps[t - NPRE]
                        emit_av(pr, hl, j)
                        if j % 4 == 3:
                            c = j // 4
                            emit_norm(pr, hl, c)
                            if hl == 1:
                                emit_stage(pr, c)
                                if c == 3:
                                    emit_a2a(pr)
                                    if pr == 0:
                                        for jj in range(4):
                                            recv(0, jj, nc.sync)
                        if t == NPRE + 4:
                            load_wp()

                    # ---- projection ----
                    # pass 1 (pair-0 sources) runs while A2A#1 flies
                    for jj in range(4):
                        recv(1, jj, nc.scalar)
                    partials = {}
                    for m in range(4):
                        pp = pssc.tile([128, 1024], f32, tag="sc",
                                       name="pp")
                        for n in range(2):
                            for kt in range(4):
                                nc.tensor.matmul(
                                    pp[:, n * 512:(n + 1) * 512],
                                    pis[kt][:, m * 128:(m + 1) * 128],
                                    wps[kt][:, n * 512:(n + 1) * 512],
                                    start=(kt == 0), stop=(kt == 3))
                        so = pj_pool.tile([128, 1024], bf16, tag="so",
                                          name="so")
                        nc.vector.tensor_add(so[:], pp[:], bp_bc[:])
                        partials[m] = so

                    for m in range(4):
                        pp = pssc.tile([128, 1024], f32, tag="sc",
                                       name="pp")
                        for n in range(2):
                            for kt in range(4, 8):
                                nc.tensor.matmul(
                                    pp[:, n * 512:(n + 1) * 512],
                                    pis[kt][:, m * 128:(m + 1) * 128],
                                    wps[kt][:, n * 512:(n + 1) * 512],
                                    start=(kt == 4), stop=(kt == 7))
                        so2 = pj_pool.tile([128, 1024], f32, tag="so2",
                                           name="so2")
                        nc.vector.tensor_add(so2[:], pp[:],
                                             partials[m][:])
                        oeng = nc.sync if m % 2 == 0 else nc.scalar
                        oeng.dma_start(out=out_p[m * 128:(m + 1) * 128, :],
                                       in_=so2[:])

                    if debug_taps:
                        for p in range(2):
                            nc.sync.dma_start(out=dbg["mt"][p],
                                              in_=mts[p][:])
                        for i2 in range(8):
                            nc.sync.dma_start(out=dbg["pi"][i2],
                                              in_=pis[i2][:])

    nc.compile()
    return nc


def _get_nc(debug_taps=False):
    key = debug_taps
    if key not in _NC_CACHE:
        _NC_CACHE[key] = _build_nc(debug_taps)
    return _NC_CACHE[key]


def _prep_in_maps(hidden_state, W_attn, b_attn, W_proj, b_proj):
    import ml_dtypes
    bf16 = ml_dtypes.bfloat16

    hidden_state = np.asarray(hidden_state, dtype=np.float32)
    W_attn = np.asarray(W_attn, dtype=np.float32)
    b_attn = np.asarray(b_attn, dtype=np.float32)
    W_proj = np.asarray(W_proj, dtype=np.float32)
    b_proj = np.asarray(b_proj, dtype=np.float32)

    # W_proj row permutation: per pair p, per source core j in group:
    # heads (4j+2p, 4j+2p+1)
    row_order = []
    for p in range(2):
        for j in range(4):
            for hh in (4 * j + 2 * p, 4 * j + 2 * p + 1):
                row_order.extend(range(hh * HD, (hh + 1) * HD))
    wp_perm = np.ascontiguousarray(W_proj[row_order, :]).astype(bf16)
    bp = np.ascontiguousarray(b_proj.reshape(1, D))

    xts = [np.ascontiguousarray(hidden_state[g].T).astype(bf16)
           for g in range(2)]

    in_maps = []
    for c in range(NCORES):
        g, j = c // 4, c % 4
        heads = [4 * j + i for i in range(4)]
        # wqk cols: Q(h0),Q(h1) | Q(h2),Q(h3) | K(h0),K(h1) | K(h2),K(h3)
        wqk = np.concatenate(
            [W_attn[:, h * HD:(h + 1) * HD] for h in heads]
            + [W_attn[:, D + h * HD:D + (h + 1) * HD] for h in heads],
            axis=1).astype(bf16)
        bqk = np.concatenate(
            [b_attn[h * HD:(h + 1) * HD] for h in heads]
            + [b_attn[D + h * HD:D + (h + 1) * HD] for h in heads])
        bqk = np.ascontiguousarray(bqk.reshape(4, 128).T)  # [128, 4]
        # V augmented with a ones column per head
        wv = np.zeros((D, 264), np.float32)
        bv = np.zeros((1, 264), np.float32)
        for i, h in enumerate(heads):
            wv[:, i * 66:i * 66 + 64] = \
                W_attn[:, 2 * D + h * HD:2 * D + (h + 1) * HD]
            bv[0, i * 66:i * 66 + 64] = \
                b_attn[2 * D + h * HD:2 * D + (h + 1) * HD]
            bv[0, i * 66 + 64] = 1.0
        gmask = np.zeros((128, 8), np.float32)
        gmask[:, 4 * g:4 * g + 4] = 1.0
        in_maps.append({
            "xt": xts[g],
            "wqk": np.ascontiguousarray(wqk),
            "wv": np.ascontiguousarray(wv.astype(bf16)),
            "wp": wp_perm,
            "bqk": bqk.astype(np.float32),
            "bv": bv,
            "bp": bp,
            "gmask": gmask,
        })
    return in_maps


def _run(in_maps, debug_taps=False, trace=False, tmpdir=None):
    from concourse.bass_utils import run_bass_kernel_spmd
    nc = _get_nc(debug_taps)
    return run_bass_kernel_spmd(nc, in_maps, core_ids=list(range(NCORES)),
                                trace=trace, tmpdir=tmpdir)


def kernel(hidden_state, W_attn, b_attn, W_proj, b_proj):
    in_maps = _prep_in_maps(hidden_state, W_attn, b_attn, W_proj, b_proj)
    res = _run(in_maps, trace=bool(os.environ.get("BASS_KERNEL_TRACE")),
               tmpdir=os.environ.get("BASS_KERNEL_TRACE_DIR") or None)
    out = np.empty((2, S, D), np.float32)
    for c in range(NCORES):
        out[c // 4, (c % 4) * SQ:(c % 4 + 1) * SQ] = res.results[c]["out"]
    if res.exec_time_ns is not None:
        kernel.last_exec_time_ns = res.exec_time_ns
    return out


kernel.last_exec_time_ns = None
